# revision 5
# baseline (speedup 1.0000x reference)
"""Sparse masked attention layer for Trainium2, sharded over 8 NeuronCores.

Strategy
--------
The reference masks attention columns (keys) not in ``mask_ind`` with -inf
before softmax and zeroes rows (queries) not in ``mask_ind`` after softmax.
Both facts mean only the ~63% of token positions present in ``mask_ind``
participate at all: rows absent from the set produce exactly ``bproj`` in
the output.  So the host compacts each batch down to its kept token set,
the device runs *dense* attention on the compacted tokens (exactly equal
to the reference's masked softmax), and the host scatters results back,
filling non-kept rows with ``bproj``.

Sharding: core = (batch b, head-group g) -> 4 batches x 2 groups of 8
heads.  Each core computes q/k/v projections for its 8 heads from the
(replicated per-batch) compacted activations, attention per head, and its
partial contribution to the output projection (rows g*512:(g+1)*512 of
Wproj).  The two partials of a batch are summed on the host (D-sharded
matmul reduce) and bproj is added there.

Device layouts (per core, Cp = padded kept-token count):
  xT   [D, Cp]   compacted activations, transposed (host-side transpose)
  qkT  [128, 8, Cp] sbuf: chunks 0-3 = q features (512), 4-7 = k features
  v    [128, NC, 8, 65] sbuf: per c-chunk, per head: 64 v-features plus a
       "keep" column (1.0 for real tokens, 0.0 for padding).  The keep
       column makes the attention matmul compute the softmax denominator
       for free (row 64 of the AV output), with padded slots excluded.
  S^T  per head: psum [128 kept-k, q] = kT^T @ qT (K=64); exp via ACT with
       scale=1/8 fused.  Layout is transposed so P^T feeds the AV matmul
       directly as the moving operand (no transposes anywhere).
  attnT [64, 8, Cp] normalized attention output, transposed - exactly the
       lhsT layout the output projection needs.

All matmuls run in float32r (full-rate PE, ~1e-4 relative accuracy).
"""

import math

import numpy as np

B, C, D, H = 4, 2048, 1024, 16
HD = D // H          # 64
HPC = H // 2         # 8 heads per core
FQ = HPC * HD        # 512 per-core q/k/v feature count
N_CORES = 8

_NC_CACHE = {}


def _chunks(total, step):
    return [(i, min(step, total - i)) for i in range(0, total, step)]


def _build_nc(Cp):
    import concourse.mybir as mybir
    import concourse.tile as tile
    from concourse import bacc

    f32 = mybir.dt.float32
    f32r = mybir.dt.float32r
    Exp = mybir.ActivationFunctionType.Exp

    NC = Cp // 128       # kept-token chunks of 128
    KD = D // 128        # 8 contraction chunks for the projections
    n512 = _chunks(Cp, 512)
    # q-dimension groups for attention: <= 2 groups, each <= 1024 wide
    half = ((NC + 1) // 2) * 128
    qgroups = _chunks(Cp, half)
    qg_max = max(sz for _, sz in qgroups)

    nc = bacc.Bacc()
    xT = nc.dram_tensor("xT", [D, Cp], f32r, kind="ExternalInput")
    wqk = nc.dram_tensor("wqk", [D, 2 * FQ], f32r, kind="ExternalInput")
    bqk = nc.dram_tensor("bqk", [1, 2 * FQ], f32r, kind="ExternalInput")
    wv = nc.dram_tensor("wv", [D, FQ], f32r, kind="ExternalInput")
    bv = nc.dram_tensor("bv", [1, FQ], f32r, kind="ExternalInput")
    wp = nc.dram_tensor("wp", [FQ, D], f32r, kind="ExternalInput")
    keep = nc.dram_tensor("keep", [128, NC], f32, kind="ExternalInput")
    keepr = nc.dram_tensor("keepr", [128, NC], f32r, kind="ExternalInput")
    onesd = nc.dram_tensor("ones", [1, Cp], f32r, kind="ExternalInput")
    outT = nc.dram_tensor("outT", [D, Cp], f32, kind="ExternalOutput")

    with tile.TileContext(nc) as tc:
        with tc.tile_pool(name="qkv", bufs=1) as p_qkv:
            qkT = p_qkv.tile([128, 8, Cp], f32r)
            vsb = p_qkv.tile([128, NC, HPC, HD + 1], f32r)

            # ---------------- phase A: projections ----------------
            with (
                tc.tile_pool(name="inp", bufs=1) as p_in,
                tc.tile_pool(name="psA", bufs=3, space="PSUM") as psA,
            ):
                xTs = p_in.tile([128, KD, Cp], f32r)
                nc.sync.dma_start(xTs[:], xT[:].rearrange("(c p) n -> p c n", p=128))
                wqks = p_in.tile([128, KD, 2 * FQ], f32r)
                nc.sync.dma_start(wqks[:], wqk[:].rearrange("(c p) n -> p c n", p=128))
                wvs = p_in.tile([128, KD, FQ], f32r)
                nc.sync.dma_start(wvs[:], wv[:].rearrange("(c p) n -> p c n", p=128))
                bqks = p_in.tile([1, 2 * FQ], f32r)
                nc.sync.dma_start(bqks[:], bqk[:])
                bvs = p_in.tile([1, FQ], f32r)
                nc.sync.dma_start(bvs[:], bv[:])
                keeps = p_in.tile([128, NC], f32)
                nc.sync.dma_start(keeps[:], keep[:])
                keeprs = p_in.tile([128, NC], f32r)
                nc.sync.dma_start(keeprs[:], keepr[:])
                ones = p_in.tile([1, Cp], f32r)
                nc.sync.dma_start(ones[:], onesd[:])

                # qkT[f, c] = (x @ Wqk + bqk)^T ; K=1 tail matmul adds the bias
                for m in range(8):
                    for n0, nsz in n512:
                        ps = psA.tile([128, 512], f32, tag="psA")
                        for k in range(KD):
                            nc.tensor.matmul(
                                ps[:, :nsz],
                                wqks[:, k, m * 128:(m + 1) * 128],
                                xTs[:, k, n0:n0 + nsz],
                                start=(k == 0), stop=False,
                            )
                        nc.tensor.matmul(
                            ps[:, :nsz],
                            bqks[0:1, m * 128:(m + 1) * 128],
                            ones[0:1, n0:n0 + nsz],
                            start=False, stop=True,
                        )
                        nc.vector.tensor_copy(qkT[:, m, n0:n0 + nsz], ps[:, :nsz])

                # v[c, f] = (x @ Wv + bv) * keep[c]; keep col = keep[c]
                for j in range(HPC):
                    nc.vector.tensor_copy(vsb[:, :, j, HD:HD + 1], keeprs[:])
                for c in range(NC):
                    ps = psA.tile([128, 512], f32, tag="psA")
                    for k in range(KD):
                        nc.tensor.matmul(
                            ps[:],
                            xTs[:, k, c * 128:(c + 1) * 128],
                            wvs[:, k, :],
                            start=(k == 0), stop=False,
                        )
                    nc.tensor.matmul(
                        ps[:], ones[0:1, c * 128:(c + 1) * 128], bvs[0:1, :],
                        start=False, stop=True,
                    )
                    nc.vector.tensor_scalar_mul(
                        vsb[:, c, :, 0:HD], ps[:], keeps[:, c:c + 1]
                    )

            # ---------------- phases B+C ----------------
            with (
                tc.tile_pool(name="att", bufs=2) as p_att,
                tc.tile_pool(name="pT", bufs=3) as p_pT,
                tc.tile_pool(name="attnT", bufs=1) as p_attnT,
                tc.tile_pool(name="wpp", bufs=1) as p_wp,
                tc.tile_pool(name="outs", bufs=3) as p_out,
            ):
                attnT = p_attnT.tile([64, HPC, Cp], f32r)
                wps = p_wp.tile([64, HPC, D], f32r)
                nc.sync.dma_start(wps[:], wp[:].rearrange("(c p) n -> p c n", p=64))

                # phase B: attention.  Head pairs share the PE via row
                # tiling (even head in array rows 0-63, odd in 64-127).
                with (
                    tc.tile_pool(name="psS", bufs=2, space="PSUM") as psS,
                    tc.tile_pool(name="psAV", bufs=2, space="PSUM") as psAV,
                ):
                    for hp in range(4):
                        heads = (2 * hp, 2 * hp + 1)
                        for q0, qsz in qgroups:
                            avs = []
                            for hi, h in enumerate(heads):
                                avs.append(psAV.tile([65, qg_max], f32, tag="av",
                                                     name=f"av_{hp}_{q0}_{hi}"))
                            for kc in range(NC):
                                sss, pTs = [], []
                                for hi, h in enumerate(heads):
                                    lo = hi * 64
                                    ss = psS.tile([128, qg_max], f32, tag="ss")
                                    for s0, ssz in _chunks(qsz, 512):
                                        nc.tensor.matmul(
                                            ss[:, s0:s0 + ssz],
                                            qkT[lo:lo + 64, 4 + hp, kc * 128:(kc + 1) * 128],
                                            qkT[lo:lo + 64, hp, q0 + s0:q0 + s0 + ssz],
                                            start=True, stop=True,
                                        )
                                    sss.append(ss)
                                for hi, h in enumerate(heads):
                                    pT = p_pT.tile([128, qg_max], f32r, tag="pT")
                                    nc.scalar.activation(
                                        pT[:, :qsz], sss[hi][:, :qsz], Exp, scale=0.125
                                    )
                                    pTs.append(pT)
                                for hi, h in enumerate(heads):
                                    for s0, ssz in _chunks(qsz, 512):
                                        nc.tensor.matmul(
                                            avs[hi][:, s0:s0 + ssz],
                                            vsb[:, kc, h, :],
                                            pTs[hi][:, s0:s0 + ssz],
                                            start=(kc == 0), stop=(kc == NC - 1),
                                        )
                            for hi, h in enumerate(heads):
                                av = avs[hi]
                                rec = p_att.tile([128, qg_max], f32, tag="rec")
                                nc.vector.reciprocal(rec[0:1, :qsz], av[64:65, :qsz])
                                bcv = p_att.tile([64, qg_max], f32, tag="bc")
                                nc.gpsimd.partition_broadcast(
                                    bcv[:, :qsz], rec[0:1, :qsz], channels=64
                                )
                                nc.vector.tensor_mul(
                                    attnT[:, h, q0:q0 + qsz],
                                    av[0:64, :qsz],
                                    bcv[:, :qsz],
                                )

                # phase C: output projection partial, transposed out
                with tc.tile_pool(name="psC", bufs=2, space="PSUM") as psC:
                    for m in range(8):
                        for n0, nsz in n512:
                            ps = psC.tile([128, 512], f32, tag="psC")
                            for j in range(HPC):
                                nc.tensor.matmul(
                                    ps[:, :nsz],
                                    wps[:, j, m * 128:(m + 1) * 128],
                                    attnT[:, j, n0:n0 + nsz],
                                    start=(j == 0), stop=(j == HPC - 1),
                                )
                            st = p_out.tile([128, 512], f32, tag="st")
                            nc.vector.tensor_copy(st[:, :nsz], ps[:, :nsz])
                            nc.sync.dma_start(
                                outT[m * 128:(m + 1) * 128, n0:n0 + nsz], st[:, :nsz]
                            )

    nc.finalize()
    return nc


def _get_nc(Cp):
    if Cp not in _NC_CACHE:
        _NC_CACHE[Cp] = _build_nc(Cp)
    return _NC_CACHE[Cp]


def kernel(x, mask_ind, Wqkv, bqkv, Wproj, bproj, **_unused):
    from concourse.bass_utils import run_bass_kernel_spmd

    x = np.asarray(x, dtype=np.float32)
    mask_ind = np.asarray(mask_ind)
    Wqkv = np.asarray(Wqkv, dtype=np.float32)
    bqkv = np.asarray(bqkv, dtype=np.float32)
    Wproj = np.asarray(Wproj, dtype=np.float32)
    bproj = np.asarray(bproj, dtype=np.float32)

    # kept-token sets per batch (matches reference _keep_mask semantics)
    idx = []
    for b in range(B):
        mi = mask_ind[b]
        mi = mi[mi >= 0]
        mi = np.clip(mi, 0, C - 1)
        idx.append(np.unique(mi).astype(np.int64))
    nmax = max(len(u) for u in idx)
    Cp = max(128, ((nmax + 127) // 128) * 128)
    NC = Cp // 128

    nc = _get_nc(Cp)

    in_maps = []
    for core in range(N_CORES):
        b, g = core // 2, core % 2
        u = idx[b]
        n = len(u)
        xk = np.zeros((Cp, D), dtype=np.float32)
        xk[:n] = x[b, u]
        keep = np.zeros(Cp, dtype=np.float32)
        keep[:n] = 1.0
        qs, ks, vs = g * FQ, D + g * FQ, 2 * D + g * FQ
        wqk = np.concatenate(
            [Wqkv[:, qs:qs + FQ], Wqkv[:, ks:ks + FQ]], axis=1
        )
        bqk = np.concatenate([bqkv[qs:qs + FQ], bqkv[ks:ks + FQ]])
        in_maps.append({
            "xT": np.ascontiguousarray(xk.T),
            "wqk": np.ascontiguousarray(wqk),
            "bqk": bqk.reshape(1, -1),
            "wv": np.ascontiguousarray(Wqkv[:, vs:vs + FQ]),
            "bv": bqkv[vs:vs + FQ].reshape(1, -1).copy(),
            "wp": np.ascontiguousarray(Wproj[g * FQ:(g + 1) * FQ, :]),
            "keep": np.ascontiguousarray(keep.reshape(NC, 128).T),
            "keepr": np.ascontiguousarray(keep.reshape(NC, 128).T),
            "ones": np.ones((1, Cp), dtype=np.float32),
        })

    global _last_in_maps
    _last_in_maps = in_maps
    res = run_bass_kernel_spmd(nc, in_maps, core_ids=list(range(N_CORES)))

    out = np.broadcast_to(bproj, (B, C, D)).copy()
    for b in range(B):
        u = idx[b]
        n = len(u)
        comb = res.results[2 * b]["outT"] + res.results[2 * b + 1]["outT"]
        out[b, u] += comb.T[:n]
    return out


# revision 7
# speedup vs baseline: 1.1411x; 1.1411x over previous
"""Sparse masked attention layer for Trainium2, sharded over 8 NeuronCores.

Strategy
--------
The reference masks attention columns (keys) not in ``mask_ind`` with -inf
before softmax and zeroes rows (queries) not in ``mask_ind`` after softmax.
Both facts mean only the ~63% of token positions present in ``mask_ind``
participate at all: rows absent from the set produce exactly ``bproj`` in
the output.  So the host compacts each batch down to its kept token set,
the device runs *dense* attention on the compacted tokens (exactly equal
to the reference's masked softmax), and the host scatters results back,
filling non-kept rows with ``bproj``.

Sharding: core = (batch b, head-group g) -> 4 batches x 2 groups of 8
heads.  Each core computes q/k/v projections for its 8 heads from the
(replicated per-batch) compacted activations, attention per head, and its
partial contribution to the output projection (rows g*512:(g+1)*512 of
Wproj).  The two partials of a batch are summed on the host (D-sharded
matmul reduce) and bproj is added there.

Device layouts (per core, Cp = padded kept-token count):
  xT   [D, Cp]   compacted activations, transposed (host-side transpose)
  qkT  [128, 8, Cp] sbuf: chunks 0-3 = q features (512), 4-7 = k features
  v    [128, NC, 8, 65] sbuf: per c-chunk, per head: 64 v-features plus a
       "keep" column (1.0 for real tokens, 0.0 for padding).  The keep
       column makes the attention matmul compute the softmax denominator
       for free (row 64 of the AV output), with padded slots excluded.
  S^T  per head: psum [128 kept-k, q] = kT^T @ qT (K=64); exp via ACT with
       scale=1/8 fused.  Layout is transposed so P^T feeds the AV matmul
       directly as the moving operand (no transposes anywhere).
  attnT [64, 8, Cp] normalized attention output, transposed - exactly the
       lhsT layout the output projection needs.

All matmuls run in float32r (full-rate PE, ~1e-4 relative accuracy).
"""

import math

import numpy as np

B, C, D, H = 4, 2048, 1024, 16
HD = D // H          # 64
HPC = H // 2         # 8 heads per core
FQ = HPC * HD        # 512 per-core q/k/v feature count
N_CORES = 8

_NC_CACHE = {}


def _chunks(total, step):
    return [(i, min(step, total - i)) for i in range(0, total, step)]


def _build_nc(Cp):
    import concourse.mybir as mybir
    import concourse.tile as tile
    from concourse import bacc

    f32 = mybir.dt.float32
    f32r = mybir.dt.float32r
    Exp = mybir.ActivationFunctionType.Exp
    Ln = mybir.ActivationFunctionType.Ln

    NC = Cp // 128       # kept-token chunks of 128
    KD = D // 128        # 8 contraction chunks for the projections
    n512 = _chunks(Cp, 512)
    # q-dimension groups for attention: 512 wide (1 PSUM bank each)
    qgroups = _chunks(Cp, 512)
    qg_max = max(sz for _, sz in qgroups)

    nc = bacc.Bacc()
    xT = nc.dram_tensor("xT", [D, Cp], f32r, kind="ExternalInput")
    wqk = nc.dram_tensor("wqk", [D, 2 * FQ], f32r, kind="ExternalInput")
    bqk = nc.dram_tensor("bqk", [1, 2 * FQ], f32r, kind="ExternalInput")
    wv = nc.dram_tensor("wv", [D, FQ], f32r, kind="ExternalInput")
    bv = nc.dram_tensor("bv", [1, FQ], f32r, kind="ExternalInput")
    wp = nc.dram_tensor("wp", [FQ, D], f32r, kind="ExternalInput")
    keep = nc.dram_tensor("keep", [128, NC], f32, kind="ExternalInput")
    keepr = nc.dram_tensor("keepr", [128, NC], f32r, kind="ExternalInput")
    onesd = nc.dram_tensor("ones", [1, Cp], f32r, kind="ExternalInput")
    outT = nc.dram_tensor("outT", [D, Cp], f32, kind="ExternalOutput")

    with tile.TileContext(nc) as tc:
        with tc.tile_pool(name="qkv", bufs=1) as p_qkv:
            qkT = p_qkv.tile([128, 8, Cp], f32r)
            vsb = p_qkv.tile([128, NC, HPC, HD + 1], f32r)

            # ---------------- phase A: projections ----------------
            with (
                tc.tile_pool(name="inp", bufs=1) as p_in,
                tc.tile_pool(name="psA", bufs=3, space="PSUM") as psA,
            ):
                xTs = p_in.tile([128, KD, Cp], f32r)
                nc.sync.dma_start(xTs[:], xT[:].rearrange("(c p) n -> p c n", p=128))
                wqks = p_in.tile([128, KD, 2 * FQ], f32r)
                nc.sync.dma_start(wqks[:], wqk[:].rearrange("(c p) n -> p c n", p=128))
                wvs = p_in.tile([128, KD, FQ], f32r)
                nc.sync.dma_start(wvs[:], wv[:].rearrange("(c p) n -> p c n", p=128))
                bqks = p_in.tile([1, 2 * FQ], f32r)
                nc.sync.dma_start(bqks[:], bqk[:])
                bvs = p_in.tile([1, FQ], f32r)
                nc.sync.dma_start(bvs[:], bv[:])
                keeps = p_in.tile([128, NC], f32)
                nc.sync.dma_start(keeps[:], keep[:])
                keeprs = p_in.tile([128, NC], f32r)
                nc.sync.dma_start(keeprs[:], keepr[:])
                ones = p_in.tile([1, Cp], f32r)
                nc.sync.dma_start(ones[:], onesd[:])

                # qkT[f, c] = (x @ Wqk + bqk)^T ; K=1 tail matmul adds the bias
                for m in range(8):
                    for n0, nsz in n512:
                        ps = psA.tile([128, 512], f32, tag="psA")
                        for k in range(KD):
                            nc.tensor.matmul(
                                ps[:, :nsz],
                                wqks[:, k, m * 128:(m + 1) * 128],
                                xTs[:, k, n0:n0 + nsz],
                                start=(k == 0), stop=False,
                            )
                        nc.tensor.matmul(
                            ps[:, :nsz],
                            bqks[0:1, m * 128:(m + 1) * 128],
                            ones[0:1, n0:n0 + nsz],
                            start=False, stop=True,
                        )
                        nc.vector.tensor_copy(qkT[:, m, n0:n0 + nsz], ps[:, :nsz])

                # v[c, f] = (x @ Wv + bv) * keep[c]; keep col = keep[c]
                for j in range(HPC):
                    nc.vector.tensor_copy(vsb[:, :, j, HD:HD + 1], keeprs[:])
                for c in range(NC):
                    ps = psA.tile([128, 512], f32, tag="psA")
                    for k in range(KD):
                        nc.tensor.matmul(
                            ps[:],
                            xTs[:, k, c * 128:(c + 1) * 128],
                            wvs[:, k, :],
                            start=(k == 0), stop=False,
                        )
                    nc.tensor.matmul(
                        ps[:], ones[0:1, c * 128:(c + 1) * 128], bvs[0:1, :],
                        start=False, stop=True,
                    )
                    nc.vector.tensor_scalar_mul(
                        vsb[:, c, :, 0:HD], ps[:], keeps[:, c:c + 1]
                    )

            # ---------------- phases B+C ----------------
            with (
                tc.tile_pool(name="att", bufs=2) as p_att,
                tc.tile_pool(name="pT", bufs=3) as p_pT,
                tc.tile_pool(name="attnT", bufs=1) as p_attnT,
                tc.tile_pool(name="wpp", bufs=1) as p_wp,
                tc.tile_pool(name="outs", bufs=3) as p_out,
            ):
                attnT = p_attnT.tile([64, HPC, Cp], f32r)
                wps = p_wp.tile([64, HPC, D], f32r)
                nc.sync.dma_start(wps[:], wp[:].rearrange("(c p) n -> p c n", p=64))

                # phase B: attention.  Head pairs share the PE via row
                # tiling (even head in array rows 0-63, odd in 64-127).
                with (
                    tc.tile_pool(name="psS", bufs=3, space="PSUM") as psS,
                    tc.tile_pool(name="psAV", bufs=4, space="PSUM") as psAV,
                ):
                    for hp in range(4):
                        heads = (2 * hp, 2 * hp + 1)
                        for q0, qsz in qgroups:
                            avs = []
                            for hi, h in enumerate(heads):
                                avs.append(psAV.tile([65, qg_max], f32, tag="av",
                                                     name=f"av_{hp}_{q0}_{hi}"))
                            for kc in range(NC):
                                sss, pTs = [], []
                                for hi, h in enumerate(heads):
                                    lo = hi * 64
                                    ss = psS.tile([128, qg_max], f32, tag="ss")
                                    for s0, ssz in _chunks(qsz, 512):
                                        nc.tensor.matmul(
                                            ss[:, s0:s0 + ssz],
                                            qkT[lo:lo + 64, 4 + hp, kc * 128:(kc + 1) * 128],
                                            qkT[lo:lo + 64, hp, q0 + s0:q0 + s0 + ssz],
                                            start=True, stop=True,
                                        )
                                    sss.append(ss)
                                for hi, h in enumerate(heads):
                                    pT = p_pT.tile([128, qg_max], f32r, tag="pT")
                                    nc.scalar.activation(
                                        pT[:, :qsz], sss[hi][:, :qsz], Exp, scale=0.125
                                    )
                                    pTs.append(pT)
                                for hi, h in enumerate(heads):
                                    for s0, ssz in _chunks(qsz, 512):
                                        nc.tensor.matmul(
                                            avs[hi][:, s0:s0 + ssz],
                                            vsb[:, kc, h, :],
                                            pTs[hi][:, s0:s0 + ssz],
                                            start=(kc == 0), stop=(kc == NC - 1),
                                        )
                            for hi, h in enumerate(heads):
                                av = avs[hi]
                                # 1/denom as exp(-ln(denom)) on ACT: stays in
                                # the natural_log_exp table set, ~7x faster
                                # than single-lane DVE reciprocal.
                                lg = p_att.tile([1, qg_max], f32, tag="lg")
                                nc.scalar.activation(lg[0:1, :qsz], av[64:65, :qsz],
                                                     Ln)
                                rec = p_att.tile([1, qg_max], f32, tag="rec")
                                nc.scalar.activation(rec[0:1, :qsz], lg[0:1, :qsz],
                                                     Exp, scale=-1.0)
                                bcv = p_att.tile([64, qg_max], f32, tag="bc")
                                nc.gpsimd.partition_broadcast(
                                    bcv[:, :qsz], rec[0:1, :qsz], channels=64
                                )
                                nc.vector.tensor_mul(
                                    attnT[:, h, q0:q0 + qsz],
                                    av[0:64, :qsz],
                                    bcv[:, :qsz],
                                )

                # phase C: output projection partial, transposed out
                with tc.tile_pool(name="psC", bufs=2, space="PSUM") as psC:
                    for m in range(8):
                        for n0, nsz in n512:
                            ps = psC.tile([128, 512], f32, tag="psC")
                            for j in range(HPC):
                                nc.tensor.matmul(
                                    ps[:, :nsz],
                                    wps[:, j, m * 128:(m + 1) * 128],
                                    attnT[:, j, n0:n0 + nsz],
                                    start=(j == 0), stop=(j == HPC - 1),
                                )
                            st = p_out.tile([128, 512], f32, tag="st")
                            nc.vector.tensor_copy(st[:, :nsz], ps[:, :nsz])
                            nc.sync.dma_start(
                                outT[m * 128:(m + 1) * 128, n0:n0 + nsz], st[:, :nsz]
                            )

    nc.finalize()
    return nc


def _get_nc(Cp):
    if Cp not in _NC_CACHE:
        _NC_CACHE[Cp] = _build_nc(Cp)
    return _NC_CACHE[Cp]


def kernel(x, mask_ind, Wqkv, bqkv, Wproj, bproj, **_unused):
    from concourse.bass_utils import run_bass_kernel_spmd

    x = np.asarray(x, dtype=np.float32)
    mask_ind = np.asarray(mask_ind)
    Wqkv = np.asarray(Wqkv, dtype=np.float32)
    bqkv = np.asarray(bqkv, dtype=np.float32)
    Wproj = np.asarray(Wproj, dtype=np.float32)
    bproj = np.asarray(bproj, dtype=np.float32)

    # kept-token sets per batch (matches reference _keep_mask semantics)
    idx = []
    for b in range(B):
        mi = mask_ind[b]
        mi = mi[mi >= 0]
        mi = np.clip(mi, 0, C - 1)
        idx.append(np.unique(mi).astype(np.int64))
    nmax = max(len(u) for u in idx)
    Cp = max(128, ((nmax + 127) // 128) * 128)
    NC = Cp // 128

    nc = _get_nc(Cp)

    in_maps = []
    for core in range(N_CORES):
        b, g = core // 2, core % 2
        u = idx[b]
        n = len(u)
        xk = np.zeros((Cp, D), dtype=np.float32)
        xk[:n] = x[b, u]
        keep = np.zeros(Cp, dtype=np.float32)
        keep[:n] = 1.0
        qs, ks, vs = g * FQ, D + g * FQ, 2 * D + g * FQ
        wqk = np.concatenate(
            [Wqkv[:, qs:qs + FQ], Wqkv[:, ks:ks + FQ]], axis=1
        )
        bqk = np.concatenate([bqkv[qs:qs + FQ], bqkv[ks:ks + FQ]])
        in_maps.append({
            "xT": np.ascontiguousarray(xk.T),
            "wqk": np.ascontiguousarray(wqk),
            "bqk": bqk.reshape(1, -1),
            "wv": np.ascontiguousarray(Wqkv[:, vs:vs + FQ]),
            "bv": bqkv[vs:vs + FQ].reshape(1, -1).copy(),
            "wp": np.ascontiguousarray(Wproj[g * FQ:(g + 1) * FQ, :]),
            "keep": np.ascontiguousarray(keep.reshape(NC, 128).T),
            "keepr": np.ascontiguousarray(keep.reshape(NC, 128).T),
            "ones": np.ones((1, Cp), dtype=np.float32),
        })

    global _last_in_maps
    _last_in_maps = in_maps
    res = run_bass_kernel_spmd(nc, in_maps, core_ids=list(range(N_CORES)))

    out = np.broadcast_to(bproj, (B, C, D)).copy()
    for b in range(B):
        u = idx[b]
        n = len(u)
        comb = res.results[2 * b]["outT"] + res.results[2 * b + 1]["outT"]
        out[b, u] += comb.T[:n]
    return out
